# revision 12
# baseline (speedup 1.0000x reference)
"""AFT-Full distributed Trainium2 kernel.

Reference computation (B=8, T=4096, D=512, H=64):
    Q = x @ wq.T + bq ; K = x @ wk.T + bk ; V = x @ wv.T + bv      [B,T,H]
    ew  = exp(wbias)                                               [T,T]
    num = ew @ (exp(K)*V) ; den = ew @ exp(K)                      [B,T,H]
    out = (sigmoid(Q) * num/den) @ wp.T + bp                       [B,T,D]

Sharding over 8 cores: 4 batch-groups x 2 t-groups.  Core c handles
batches {2*(c//2), 2*(c//2)+1} and output rows t in slice (c%2) of T.
Each core's work is fully independent -> no collectives; the host
scatters inputs and gathers the per-core output slices.

Per-core dataflow (all matmuls bf16 with fp32 PSUM accumulate):
  phase 1: K|V = x^T.T @ [wk^T|wv^T] per 128-row s-chunk (x^T tile is the
           stationary operand), +bias via a K=1 ones-row matmul, then
           eK = exp(K) (ACT) and eKV = eK*V (DVE) packed into
           Z[s, 0:64]=eKV, Z[s,64:128]=eK.  Q^T = wq^T.T @ x^T for the
           core's t-slice only; sigmoid applied on ACT with per-partition
           bias.
  phase 2: for each s-chunk: DMA one bf16 row-block of wbias^T
           [128 s x 2048 t], exp it in one ACT op, then 8 matmuls
           accumulate [num^T;den^T] (2 batches x 4 t-blocks of 512) in
           PSUM with Z slices as stationary operands.  wbias traffic is
           shared by both batches.
  phase 3: Yt^T = sigmoid(Q^T) * num^T * recip(den^T) (DVE), then
           out[t,:] = Yt^T.T @ wp^T + bp (bias via K=1 matmul), DMA the
           PSUM result straight to DRAM.

The s-axis order of Z rows and wbias^T rows is permuted per-core (own
t-slice first) so that the Q projection can read x^T columns [0:TPC]
uniformly across the SPMD graph; the contraction over s is invariant to
that permutation.
"""

import sys

for _p in ("/opt/trn_rl_repo", "/opt/pypackages"):
    if _p not in sys.path:
        sys.path.append(_p)

import numpy as np
import ml_dtypes

B, T, D, H = 8, 4096, 512, 64
BG, TG = 4, 2            # batch groups x t groups = 8 cores
BPC = B // BG            # batches per core
TPC = T // TG            # t rows per core
TBLK = 512               # t columns per PSUM bank
NTB = TPC // TBLK        # t blocks per core
NS = T // 128            # s chunks
NDC = D // 128           # d chunks
XCH = 1024               # x^T DMA column chunk
N_CORES = 8

_NC_CACHE = {}


def _build_module(use_bias):
    import concourse.bass as bass
    import concourse.mybir as mybir
    import concourse.tile as tile
    from concourse import bacc
    from contextlib import ExitStack

    bf16 = mybir.dt.bfloat16
    f32 = mybir.dt.float32
    Exp = mybir.ActivationFunctionType.Exp
    Sigmoid = mybir.ActivationFunctionType.Sigmoid
    mult = mybir.AluOpType.mult
    add = mybir.AluOpType.add

    nc = bacc.Bacc("TRN2", target_bir_lowering=False, debug=False,
                   num_devices=N_CORES)

    xT = nc.dram_tensor("xT", [BPC, D, T], bf16, kind="ExternalInput").ap()
    wbT = nc.dram_tensor("wbT", [T, TPC], bf16, kind="ExternalInput").ap()
    wkv = nc.dram_tensor("wkv", [D, 2 * H], bf16, kind="ExternalInput").ap()
    wqT = nc.dram_tensor("wqT", [D, H], bf16, kind="ExternalInput").ap()
    wpT = nc.dram_tensor("wpT", [H + 1, D], bf16, kind="ExternalInput").ap()
    bkv = nc.dram_tensor("bkv", [1, 2 * H], bf16, kind="ExternalInput").ap()
    bqv = nc.dram_tensor("bqv", [H, 1], f32, kind="ExternalInput").ap()
    ones = nc.dram_tensor("ones", [1, 128], bf16, kind="ExternalInput").ap()
    out = nc.dram_tensor("out", [BPC, TPC, D], f32, kind="ExternalOutput").ap()

    with tile.TileContext(nc) as tc, ExitStack() as ctx:
        wpool = ctx.enter_context(tc.tile_pool(name="wts", bufs=1))
        xpool = ctx.enter_context(
            tc.tile_pool(name="xt", bufs=BPC * NDC * (T // XCH)))
        zpool = ctx.enter_context(tc.tile_pool(name="z", bufs=BPC))
        sqpool = ctx.enter_context(tc.tile_pool(name="sq", bufs=BPC))
        ewpool = ctx.enter_context(tc.tile_pool(name="ewr", bufs=4))
        expool = ctx.enter_context(tc.tile_pool(name="ewx", bufs=8))
        ytpool = ctx.enter_context(tc.tile_pool(name="yt", bufs=2))
        tpool = ctx.enter_context(tc.tile_pool(name="tmp", bufs=4))
        opool = ctx.enter_context(tc.tile_pool(name="osb", bufs=4))
        ps = ctx.enter_context(tc.tile_pool(name="ps", bufs=8, space="PSUM"))

        # --- resident weights / constants ---
        wkv_sb = wpool.tile([128, NDC * 2 * H], bf16)   # [128, 512]
        for d in range(NDC):
            nc.sync.dma_start(wkv_sb[:, d * 2 * H:(d + 1) * 2 * H],
                              wkv[d * 128:(d + 1) * 128, :])
        wq_sb = wpool.tile([128, NDC * H], bf16)        # [128, 256]
        for d in range(NDC):
            nc.sync.dma_start(wq_sb[:, d * H:(d + 1) * H],
                              wqT[d * 128:(d + 1) * 128, :])
        wp_sb = wpool.tile([H + 1, D], bf16)            # [65, 512]
        nc.sync.dma_start(wp_sb[:, :], wpT[:, :])
        bkv_sb = wpool.tile([1, 2 * H], bf16)
        nc.sync.dma_start(bkv_sb[:, :], bkv[:, :])
        bq_sb = wpool.tile([H, 1], f32)
        nc.sync.dma_start(bq_sb[:, :], bqv[:, :])
        ones_sb = wpool.tile([1, 128], bf16)
        nc.sync.dma_start(ones_sb[:, :], ones[:, :])

        # --- phase 1: Z = [eKV | eK] per batch, sigmoid(Q^T) per batch ---
        z_sb = []
        sq_sb = []
        for b in range(BPC):
            xt_b = []
            for d in range(NDC):
                row = []
                for ch in range(T // XCH):
                    t_ = xpool.tile([128, XCH], bf16)
                    nc.gpsimd.dma_start(
                        t_[:, :],
                        xT[b, d * 128:(d + 1) * 128,
                           ch * XCH:(ch + 1) * XCH])
                    row.append(t_)
                xt_b.append(row)

            z_b = zpool.tile([128, NS * 128], bf16)
            # groups of 4 s-chunks share one PSUM bank so the ACT/DVE
            # epilogues run on 256-col batches instead of 64-col slivers
            for g in range(NS // 4):
                pkv = ps.tile([128, 512], mybir.dt.float32, tag="ps")
                for si in range(4):
                    s = g * 4 + si
                    for d in range(NDC):
                        nc.tensor.matmul(
                            pkv[:, si * 128:(si + 1) * 128],
                            lhsT=xt_b[d][(s * 128) // XCH][
                                :, (s * 128) % XCH:(s * 128) % XCH + 128],
                            rhs=wkv_sb[:, d * 128:(d + 1) * 128],
                            start=(d == 0),
                            stop=(not use_bias and d == NDC - 1))
                    if use_bias:
                        nc.tensor.matmul(
                            pkv[:, si * 128:(si + 1) * 128],
                            lhsT=ones_sb[:, :], rhs=bkv_sb[:, :],
                            start=False, stop=True)
                pk3 = pkv[:, :].rearrange("p (c k) -> p c k", c=4)
                zg3 = z_b[:, g * 512:(g + 1) * 512].rearrange(
                    "p (c k) -> p c k", c=4)
                # eK = exp(K + bk)
                nc.scalar.activation(zg3[:, :, H:2 * H], pk3[:, :, 0:H], Exp)
                # eKV = eK * (V + bv)
                nc.vector.tensor_tensor(
                    zg3[:, :, 0:H], pk3[:, :, H:2 * H], zg3[:, :, H:2 * H],
                    mult)
            z_sb.append(z_b)

            sq_b = sqpool.tile([H, TPC], mybir.dt.float32)
            for tb in range(NTB):
                pq = ps.tile([128, 512], mybir.dt.float32, tag="ps")
                for d in range(NDC):
                    nc.tensor.matmul(
                        pq[0:H, :],
                        lhsT=wq_sb[:, d * H:(d + 1) * H],
                        rhs=xt_b[d][(tb * TBLK) // XCH][
                            :, (tb * TBLK) % XCH:(tb * TBLK) % XCH + TBLK],
                        start=(d == 0), stop=(d == NDC - 1))
                nc.scalar.activation(
                    sq_b[:, tb * TBLK:(tb + 1) * TBLK], pq[0:H, :], Sigmoid,
                    bias=bq_sb[:, :])
            sq_sb.append(sq_b)

        # --- phase 2: [num^T; den^T] for 2 batches x 4 t-blocks ---
        nd = [ps.tile([128, TBLK], mybir.dt.float32, name=f"nd{i}", tag="ps")
              for i in range(BPC * NTB)]
        for s in range(NS):
            ewr = ewpool.tile([128, TPC], bf16)
            nc.sync.dma_start(ewr[:, :], wbT[s * 128:(s + 1) * 128, :])
            ewx = expool.tile([128, TPC], bf16)
            if s % 2 == 0:
                nc.scalar.activation(ewx[:, :], ewr[:, :], Exp)
            else:
                # exp(w) ~= (1 + w/2)^2 on DVE; |w| <= ~0.12 so the error
                # (~w^2/4, systematic) cancels in num/den
                th = ewpool.tile([128, TPC], bf16, name="th", tag="th")
                nc.vector.tensor_scalar(
                    th[:, :], ewr[:, :], 0.5, 1.0,
                    mybir.AluOpType.mult, mybir.AluOpType.add)
                nc.vector.tensor_tensor(ewx[:, :], th[:, :], th[:, :], mult)
            for b in range(BPC):
                for tb in range(NTB):
                    nc.tensor.matmul(
                        nd[b * NTB + tb][:, :],
                        lhsT=z_sb[b][:, s * 128:(s + 1) * 128],
                        rhs=ewx[:, tb * TBLK:(tb + 1) * TBLK],
                        start=(s == 0), stop=(s == NS - 1))

        # --- phase 3: Yt^T = sQ^T * num^T / den^T ; out = Yt^T.T@wp^T+bp ---
        for b in range(BPC):
            for tb in range(NTB):
                p = nd[b * NTB + tb]
                dcp = tpool.tile([H, TBLK], mybir.dt.float32)
                nc.scalar.copy(dcp[:, :], p[H:2 * H, :])
                rec = tpool.tile([H, TBLK], mybir.dt.float32)
                nc.vector.reciprocal_approx_fast(rec[:, :], dcp[:, :])
                tmp = tpool.tile([H, TBLK], mybir.dt.float32)
                nc.vector.tensor_tensor(tmp[:, :], p[0:H, :], rec[:, :], mult)
                yt = ytpool.tile([H + 1, TBLK], bf16)
                nc.gpsimd.memset(yt[H:H + 1, :], 1.0)
                nc.vector.tensor_tensor(
                    yt[0:H, :], tmp[:, :],
                    sq_sb[b][:, tb * TBLK:(tb + 1) * TBLK], mult)
                for c in range(TBLK // 128):
                    po = ps.tile([128, 512], mybir.dt.float32, tag="ps")
                    nc.tensor.matmul(
                        po[:, :], lhsT=yt[:, c * 128:(c + 1) * 128],
                        rhs=wp_sb[:, :], start=True, stop=True)
                    osb = opool.tile([128, 512], mybir.dt.float32)
                    if c % 2 == 0:
                        nc.scalar.copy(osb[:, :], po[:, :])
                    else:
                        nc.vector.tensor_copy(osb[:, :], po[:, :])
                    t0 = tb * TBLK + c * 128
                    nc.gpsimd.dma_start(out[b, t0:t0 + 128, :], osb[:, :])

    nc.compile()
    from concourse.bass_interp import get_hw_module
    nc.m = get_hw_module(nc.m)
    return nc


def _get_module(use_bias):
    key = ("nc", use_bias)
    if key not in _NC_CACHE:
        _NC_CACHE[key] = _build_module(use_bias)
    return _NC_CACHE[key]


def kernel(x, wq, bq, wk, bk, wv, bv, wp, bp, wbias):
    from concourse.bass_utils import run_bass_kernel_spmd

    bf16 = ml_dtypes.bfloat16
    x = np.asarray(x, np.float32)
    wbias = np.asarray(wbias, np.float32)

    # x^T per batch: [B, D, T] bf16
    xT_full = np.ascontiguousarray(x.transpose(0, 2, 1)).astype(bf16)
    # wbias^T: [s, t] bf16
    wbT_full = np.ascontiguousarray(wbias.T).astype(bf16)

    wkv_h = np.concatenate([wk.T, wv.T], axis=1).astype(bf16)      # [D, 2H]
    wqT_h = np.ascontiguousarray(wq.T).astype(bf16)                # [D, H]
    wpT_h = np.concatenate(
        [wp.T, np.asarray(bp, np.float32)[None, :]], axis=0).astype(bf16)
    bkv_h = np.concatenate([bk, bv])[None, :].astype(bf16)         # [1, 2H]
    bq_h = np.asarray(bq, np.float32)[:, None].copy()              # [H, 1]
    ones_h = np.ones((1, 128), dtype=bf16)
    use_bias = bool(np.any(bk) or np.any(bv))

    # Per t-group: s-permuted inputs (own t-slice rows first) so the SPMD
    # graph reads Q's x^T columns at [0:TPC] on every core.
    perm = {}
    for tj in range(TG):
        p = np.concatenate([
            np.arange(tj * TPC, (tj + 1) * TPC),
            np.arange(0, tj * TPC),
            np.arange((tj + 1) * TPC, T)])
        perm[tj] = p
    wbT_tj = {tj: np.ascontiguousarray(
        wbT_full[perm[tj]][:, tj * TPC:(tj + 1) * TPC]) for tj in range(TG)}

    in_maps = []
    for c in range(N_CORES):
        bi, tj = c // TG, c % TG
        in_maps.append({
            "xT": np.ascontiguousarray(
                xT_full[bi * BPC:(bi + 1) * BPC][:, :, perm[tj]]),
            "wbT": wbT_tj[tj],
            "wkv": wkv_h, "wqT": wqT_h, "wpT": wpT_h,
            "bkv": bkv_h, "bqv": bq_h, "ones": ones_h,
        })

    nc = _get_module(use_bias)
    res = run_bass_kernel_spmd(nc, in_maps, core_ids=list(range(N_CORES)))

    full = np.empty((B, T, D), dtype=np.float32)
    for c in range(N_CORES):
        bi, tj = c // TG, c % TG
        full[bi * BPC:(bi + 1) * BPC, tj * TPC:(tj + 1) * TPC, :] = \
            res.results[c]["out"]
    return full


# revision 13
# speedup vs baseline: 1.0315x; 1.0315x over previous
"""AFT-Full distributed Trainium2 kernel.

Reference computation (B=8, T=4096, D=512, H=64):
    Q = x @ wq.T + bq ; K = x @ wk.T + bk ; V = x @ wv.T + bv      [B,T,H]
    ew  = exp(wbias)                                               [T,T]
    num = ew @ (exp(K)*V) ; den = ew @ exp(K)                      [B,T,H]
    out = (sigmoid(Q) * num/den) @ wp.T + bp                       [B,T,D]

Sharding over 8 cores: 4 batch-groups x 2 t-groups.  Core c handles
batches {2*(c//2), 2*(c//2)+1} and output rows t in slice (c%2) of T.
Each core's work is fully independent -> no collectives; the host
scatters inputs and gathers the per-core output slices.

Per-core dataflow (all matmuls bf16 with fp32 PSUM accumulate):
  phase 1: K|V = x^T.T @ [wk^T|wv^T] per 128-row s-chunk (x^T tile is the
           stationary operand), +bias via a K=1 ones-row matmul, then
           eK = exp(K) (ACT) and eKV = eK*V (DVE) packed into
           Z[s, 0:64]=eKV, Z[s,64:128]=eK.  Q^T = wq^T.T @ x^T for the
           core's t-slice only; sigmoid applied on ACT with per-partition
           bias.
  phase 2: for each s-chunk: DMA one bf16 row-block of wbias^T
           [128 s x 2048 t], exp it in one ACT op, then 8 matmuls
           accumulate [num^T;den^T] (2 batches x 4 t-blocks of 512) in
           PSUM with Z slices as stationary operands.  wbias traffic is
           shared by both batches.
  phase 3: Yt^T = sigmoid(Q^T) * num^T * recip(den^T) (DVE), then
           out[t,:] = Yt^T.T @ wp^T + bp (bias via K=1 matmul), DMA the
           PSUM result straight to DRAM.

The s-axis order of Z rows and wbias^T rows is permuted per-core (own
t-slice first) so that the Q projection can read x^T columns [0:TPC]
uniformly across the SPMD graph; the contraction over s is invariant to
that permutation.
"""

import sys

for _p in ("/opt/trn_rl_repo", "/opt/pypackages"):
    if _p not in sys.path:
        sys.path.append(_p)

import numpy as np
import ml_dtypes

B, T, D, H = 8, 4096, 512, 64
BG, TG = 4, 2            # batch groups x t groups = 8 cores
BPC = B // BG            # batches per core
TPC = T // TG            # t rows per core
TBLK = 512               # t columns per PSUM bank
NTB = TPC // TBLK        # t blocks per core
NS = T // 128            # s chunks
NDC = D // 128           # d chunks
XCH = 1024               # x^T DMA column chunk
N_CORES = 8

_NC_CACHE = {}


def _build_module(use_bias):
    import concourse.bass as bass
    import concourse.mybir as mybir
    import concourse.tile as tile
    from concourse import bacc
    from contextlib import ExitStack

    bf16 = mybir.dt.bfloat16
    f32 = mybir.dt.float32
    Exp = mybir.ActivationFunctionType.Exp
    Sigmoid = mybir.ActivationFunctionType.Sigmoid
    mult = mybir.AluOpType.mult
    add = mybir.AluOpType.add

    nc = bacc.Bacc("TRN2", target_bir_lowering=False, debug=False,
                   num_devices=N_CORES)

    xT = nc.dram_tensor("xT", [BPC, D, T], bf16, kind="ExternalInput").ap()
    wbT = nc.dram_tensor("wbT", [T, TPC], bf16, kind="ExternalInput").ap()
    wkv = nc.dram_tensor("wkv", [D, 2 * H], bf16, kind="ExternalInput").ap()
    wqT = nc.dram_tensor("wqT", [D, H], bf16, kind="ExternalInput").ap()
    wpT = nc.dram_tensor("wpT", [H + 1, D], bf16, kind="ExternalInput").ap()
    bkv = nc.dram_tensor("bkv", [1, 2 * H], bf16, kind="ExternalInput").ap()
    bqv = nc.dram_tensor("bqv", [H, 1], f32, kind="ExternalInput").ap()
    ones = nc.dram_tensor("ones", [1, 128], bf16, kind="ExternalInput").ap()
    out = nc.dram_tensor("out", [BPC, TPC, D], f32, kind="ExternalOutput").ap()

    with tile.TileContext(nc) as tc, ExitStack() as ctx:
        wpool = ctx.enter_context(tc.tile_pool(name="wts", bufs=1))
        xpool = ctx.enter_context(
            tc.tile_pool(name="xt", bufs=BPC * NDC * (T // XCH)))
        zpool = ctx.enter_context(tc.tile_pool(name="z", bufs=BPC))
        sqpool = ctx.enter_context(tc.tile_pool(name="sq", bufs=BPC))
        ewpool = ctx.enter_context(tc.tile_pool(name="ewr", bufs=4))
        expool = ctx.enter_context(tc.tile_pool(name="ewx", bufs=8))
        ytpool = ctx.enter_context(tc.tile_pool(name="yt", bufs=2))
        tpool = ctx.enter_context(tc.tile_pool(name="tmp", bufs=4))
        opool = ctx.enter_context(tc.tile_pool(name="osb", bufs=4))
        ps = ctx.enter_context(tc.tile_pool(name="ps", bufs=8, space="PSUM"))

        # --- resident weights / constants ---
        wkv_sb = wpool.tile([128, NDC * 2 * H], bf16)   # [128, 512]
        for d in range(NDC):
            nc.sync.dma_start(wkv_sb[:, d * 2 * H:(d + 1) * 2 * H],
                              wkv[d * 128:(d + 1) * 128, :])
        wq_sb = wpool.tile([128, NDC * H], bf16)        # [128, 256]
        for d in range(NDC):
            nc.sync.dma_start(wq_sb[:, d * H:(d + 1) * H],
                              wqT[d * 128:(d + 1) * 128, :])
        wp_sb = wpool.tile([H + 1, D], bf16)            # [65, 512]
        nc.sync.dma_start(wp_sb[:, :], wpT[:, :])
        bkv_sb = wpool.tile([1, 2 * H], bf16)
        nc.sync.dma_start(bkv_sb[:, :], bkv[:, :])
        bq_sb = wpool.tile([H, 1], f32)
        nc.sync.dma_start(bq_sb[:, :], bqv[:, :])
        ones_sb = wpool.tile([1, 128], bf16)
        nc.sync.dma_start(ones_sb[:, :], ones[:, :])

        # --- phase 1: Z = [eKV | eK] per batch, sigmoid(Q^T) per batch ---
        z_sb = []
        sq_sb = []
        for b in range(BPC):
            xt_b = []
            for d in range(NDC):
                row = []
                for ch in range(T // XCH):
                    t_ = xpool.tile([128, XCH], bf16)
                    nc.gpsimd.dma_start(
                        t_[:, :],
                        xT[b, d * 128:(d + 1) * 128,
                           ch * XCH:(ch + 1) * XCH])
                    row.append(t_)
                xt_b.append(row)

            z_b = zpool.tile([128, NS * 128], bf16)
            # groups of 4 s-chunks share one PSUM bank so the ACT/DVE
            # epilogues run on 256-col batches instead of 64-col slivers
            for g in range(NS // 4):
                pkv = ps.tile([128, 512], mybir.dt.float32, tag="ps")
                for si in range(4):
                    s = g * 4 + si
                    for d in range(NDC):
                        nc.tensor.matmul(
                            pkv[:, si * 128:(si + 1) * 128],
                            lhsT=xt_b[d][(s * 128) // XCH][
                                :, (s * 128) % XCH:(s * 128) % XCH + 128],
                            rhs=wkv_sb[:, d * 128:(d + 1) * 128],
                            start=(d == 0),
                            stop=(not use_bias and d == NDC - 1))
                    if use_bias:
                        nc.tensor.matmul(
                            pkv[:, si * 128:(si + 1) * 128],
                            lhsT=ones_sb[:, :], rhs=bkv_sb[:, :],
                            start=False, stop=True)
                pk3 = pkv[:, :].rearrange("p (c k) -> p c k", c=4)
                zg3 = z_b[:, g * 512:(g + 1) * 512].rearrange(
                    "p (c k) -> p c k", c=4)
                # eK = exp(K + bk)
                nc.scalar.activation(zg3[:, :, H:2 * H], pk3[:, :, 0:H], Exp)
                # eKV = eK * (V + bv)
                nc.vector.tensor_tensor(
                    zg3[:, :, 0:H], pk3[:, :, H:2 * H], zg3[:, :, H:2 * H],
                    mult)
            z_sb.append(z_b)

            sq_b = sqpool.tile([H, TPC], mybir.dt.float32)
            for tb in range(NTB):
                pq = ps.tile([128, 512], mybir.dt.float32, tag="ps")
                for d in range(NDC):
                    nc.tensor.matmul(
                        pq[0:H, :],
                        lhsT=wq_sb[:, d * H:(d + 1) * H],
                        rhs=xt_b[d][(tb * TBLK) // XCH][
                            :, (tb * TBLK) % XCH:(tb * TBLK) % XCH + TBLK],
                        start=(d == 0), stop=(d == NDC - 1))
                nc.scalar.activation(
                    sq_b[:, tb * TBLK:(tb + 1) * TBLK], pq[0:H, :], Sigmoid,
                    bias=bq_sb[:, :])
            sq_sb.append(sq_b)

        # --- phase 2: [num^T; den^T] for 2 batches x 4 t-blocks ---
        nd = [ps.tile([128, TBLK], mybir.dt.float32, name=f"nd{i}", tag="ps")
              for i in range(BPC * NTB)]
        for s in range(NS):
            ewr = ewpool.tile([128, TPC], bf16)
            nc.sync.dma_start(ewr[:, :], wbT[s * 128:(s + 1) * 128, :])
            ewx = expool.tile([128, TPC], bf16)
            if True:
                nc.scalar.activation(ewx[:, :], ewr[:, :], Exp)
            else:
                # exp(w) ~= (1 + w/2)^2 on DVE; |w| <= ~0.12 so the error
                # (~w^2/4, systematic) cancels in num/den
                th = ewpool.tile([128, TPC], bf16, name="th", tag="th")
                nc.vector.tensor_scalar(
                    th[:, :], ewr[:, :], 0.5, 1.0,
                    mybir.AluOpType.mult, mybir.AluOpType.add)
                nc.vector.tensor_tensor(ewx[:, :], th[:, :], th[:, :], mult)
            for b in range(BPC):
                for tb in range(NTB):
                    nc.tensor.matmul(
                        nd[b * NTB + tb][:, :],
                        lhsT=z_sb[b][:, s * 128:(s + 1) * 128],
                        rhs=ewx[:, tb * TBLK:(tb + 1) * TBLK],
                        start=(s == 0), stop=(s == NS - 1))

        # --- phase 3: Yt^T = sQ^T * num^T / den^T ; out = Yt^T.T@wp^T+bp ---
        for b in range(BPC):
            for tb in range(NTB):
                p = nd[b * NTB + tb]
                dcp = tpool.tile([H, TBLK], mybir.dt.float32)
                nc.scalar.copy(dcp[:, :], p[H:2 * H, :])
                rec = tpool.tile([H, TBLK], mybir.dt.float32)
                nc.vector.reciprocal_approx_fast(rec[:, :], dcp[:, :])
                tmp = tpool.tile([H, TBLK], mybir.dt.float32)
                nc.vector.tensor_tensor(tmp[:, :], p[0:H, :], rec[:, :], mult)
                yt = ytpool.tile([H + 1, TBLK], bf16)
                nc.gpsimd.memset(yt[H:H + 1, :], 1.0)
                nc.vector.tensor_tensor(
                    yt[0:H, :], tmp[:, :],
                    sq_sb[b][:, tb * TBLK:(tb + 1) * TBLK], mult)
                for c in range(TBLK // 128):
                    po = ps.tile([128, 512], mybir.dt.float32, tag="ps")
                    nc.tensor.matmul(
                        po[:, :], lhsT=yt[:, c * 128:(c + 1) * 128],
                        rhs=wp_sb[:, :], start=True, stop=True)
                    osb = opool.tile([128, 512], mybir.dt.float32)
                    if c % 2 == 0:
                        nc.scalar.copy(osb[:, :], po[:, :])
                    else:
                        nc.vector.tensor_copy(osb[:, :], po[:, :])
                    t0 = tb * TBLK + c * 128
                    nc.gpsimd.dma_start(out[b, t0:t0 + 128, :], osb[:, :])

    nc.compile()
    from concourse.bass_interp import get_hw_module
    nc.m = get_hw_module(nc.m)
    return nc


def _get_module(use_bias):
    key = ("nc", use_bias)
    if key not in _NC_CACHE:
        _NC_CACHE[key] = _build_module(use_bias)
    return _NC_CACHE[key]


def kernel(x, wq, bq, wk, bk, wv, bv, wp, bp, wbias):
    from concourse.bass_utils import run_bass_kernel_spmd

    bf16 = ml_dtypes.bfloat16
    x = np.asarray(x, np.float32)
    wbias = np.asarray(wbias, np.float32)

    # x^T per batch: [B, D, T] bf16
    xT_full = np.ascontiguousarray(x.transpose(0, 2, 1)).astype(bf16)
    # wbias^T: [s, t] bf16
    wbT_full = np.ascontiguousarray(wbias.T).astype(bf16)

    wkv_h = np.concatenate([wk.T, wv.T], axis=1).astype(bf16)      # [D, 2H]
    wqT_h = np.ascontiguousarray(wq.T).astype(bf16)                # [D, H]
    wpT_h = np.concatenate(
        [wp.T, np.asarray(bp, np.float32)[None, :]], axis=0).astype(bf16)
    bkv_h = np.concatenate([bk, bv])[None, :].astype(bf16)         # [1, 2H]
    bq_h = np.asarray(bq, np.float32)[:, None].copy()              # [H, 1]
    ones_h = np.ones((1, 128), dtype=bf16)
    use_bias = bool(np.any(bk) or np.any(bv))

    # Per t-group: s-permuted inputs (own t-slice rows first) so the SPMD
    # graph reads Q's x^T columns at [0:TPC] on every core.
    perm = {}
    for tj in range(TG):
        p = np.concatenate([
            np.arange(tj * TPC, (tj + 1) * TPC),
            np.arange(0, tj * TPC),
            np.arange((tj + 1) * TPC, T)])
        perm[tj] = p
    wbT_tj = {tj: np.ascontiguousarray(
        wbT_full[perm[tj]][:, tj * TPC:(tj + 1) * TPC]) for tj in range(TG)}

    in_maps = []
    for c in range(N_CORES):
        bi, tj = c // TG, c % TG
        in_maps.append({
            "xT": np.ascontiguousarray(
                xT_full[bi * BPC:(bi + 1) * BPC][:, :, perm[tj]]),
            "wbT": wbT_tj[tj],
            "wkv": wkv_h, "wqT": wqT_h, "wpT": wpT_h,
            "bkv": bkv_h, "bqv": bq_h, "ones": ones_h,
        })

    nc = _get_module(use_bias)
    res = run_bass_kernel_spmd(nc, in_maps, core_ids=list(range(N_CORES)))

    full = np.empty((B, T, D), dtype=np.float32)
    for c in range(N_CORES):
        bi, tj = c // TG, c % TG
        full[bi * BPC:(bi + 1) * BPC, tj * TPC:(tj + 1) * TPC, :] = \
            res.results[c]["out"]
    return full
